# revision 31
# baseline (speedup 1.0000x reference)
"""Census loss (nn_CensusLoss) Trainium2 Bass kernel, 8-core data parallel.

Contract: kernel(pred, gt) -> np.float32 scalar, pred/gt f32 [8,3,512,512].
Shards batch over 8 NeuronCores (1 image pair per core).

Math per core (B=1, C=3, H=W=512, 5x5 census window, reflect pad 2):
  S = sum_{o in O24} sum_q xor_o(q),
  xor_o(q) = |1{P(m(q+o))<P(q)} - 1{G(m(q+o))<G(q)}|   (m = reflect map)
Loss = S_total / (8*3*512*512).

Device algorithm ("half-offset" decomposition, validated in fp64 on host):
only the 12 half-space offsets u are compared (fp16, DVE tensor_tensor at
2x), each full-image xor_u tile is reduced on the TensorEngine by
accumulating ones-weighted matmuls into PSUM with weight +2, plus
boundary-correction rectangle reductions (weights from a host-built
constant table, masked per partition) that exactly reproduce the reverse
offsets' reflect-boundary terms. fp16 compares only differ from fp32 via
rounding ties (~5e-6 relative on this distribution).

Layout per core: image rows banded 4-per-partition with 2-row halo; the
host supplies x[128, 24768] fp16 = for each of 6 (tensor,channel) bands,
8 padded rows (4p..4p+8) x 516 cols per partition. Output: per-column
partial sums [1,512] f32; host sums in f64 and averages across cores.
"""
import sys

if "/opt/trn_rl_repo" not in sys.path:
    sys.path.insert(0, "/opt/trn_rl_repo")

import numpy as np

H = W = 512
PAD = 2
NCORES = 8
R_PER_PART = 4  # image rows per partition
BANDW = H + 2 * PAD  # 516
BANDF = 8 * BANDW  # 4128 elems per (band, partition)
XFREE = 6 * BANDF  # 24768

O24 = [(di, dj) for di in range(-2, 3) for dj in range(-2, 3)
       if not (di == 0 and dj == 0)]
O12 = [(di, dj) for (di, dj) in O24 if (di > 0) or (di == 0 and dj > 0)]
O12SET = set(O12)


def _reflect(k, n=H):
    if k < 0:
        return -k
    if k > n - 1:
        return 2 * (n - 1) - k
    return k


def gen_terms():
    """(u, y0, y1, x0, x1, weight) rect-reduction terms over xor_u tiles.
    sum of all terms == sum_{o in O24} F(o) exactly (up to fp value ties)."""
    terms = [(u, 0, H, 0, W, 2) for u in O12]
    raw = []
    for o in O12:
        di, dj = o
        # StripA(o): subtract xor_o over {q: q+o outside img}
        if di > 0:
            terms.append((o, H - di, H, 0, W, -1))
        yhi = H - di if di > 0 else H
        if dj > 0:
            terms.append((o, 0, yhi, W - dj, W, -1))
        elif dj < 0:
            terms.append((o, 0, yhi, 0, -dj, -1))
        # StripB(o): add xor_{-o} over {q: q-o outside img}; resolve each
        # cell's reflected read to a computed tile via effective offsets.
        cells = []
        for y in range(0, di):
            for x in range(W):
                cells.append(((y, x), (_reflect(y - di) - y, _reflect(x - dj) - x)))
        xs = (list(range(0, dj)) if dj > 0
              else list(range(W + dj, W)) if dj < 0 else [])
        for x in xs:
            for y in range(max(di, 0), H):
                cells.append(((y, x), (-di, _reflect(x - dj) - x)))
        for (y, x), (ey, ex) in cells:
            if (ey, ex) == (0, 0):
                continue  # structural self-tie, contributes 0
            if (ey, ex) in O12SET:
                raw.append(((ey, ex), y, x))
            else:
                ny, nx = y + ey, x + ex
                assert 0 <= ny < H and 0 <= nx < W and (-ey, -ex) in O12SET
                raw.append(((-ey, -ex), ny, nx))
    # merge cells (with multiplicity) into rects
    from collections import Counter, defaultdict
    cellcnt = Counter()
    for u, y, x in raw:
        cellcnt[(u, y, x)] += 1
    bycol = defaultdict(list)
    for (u, y, x), mult in cellcnt.items():
        bycol[(u, mult, x)].append(y)
    rects = []
    for (u, mult, x), ys in bycol.items():
        ys = sorted(ys)
        start = prev = ys[0]
        for y in ys[1:]:
            if y == prev + 1:
                prev = y
                continue
            rects.append((u, start, prev + 1, x, x + 1, mult))
            start = prev = y
        rects.append((u, start, prev + 1, x, x + 1, mult))
    bying = defaultdict(list)
    for u, y0, y1, x0, x1, wgt in rects:
        bying[(u, y0, y1, wgt)].append(x0)
    for (u, y0, y1, wgt), xs in bying.items():
        xs = sorted(xs)
        start = prev = xs[0]
        for x in xs[1:]:
            if x == prev + 1:
                prev = x
                continue
            terms.append((u, y0, y1, start, prev + 1, wgt))
            start = prev = x
        terms.append((u, y0, y1, start, prev + 1, wgt))
    return terms


def _y_pieces(y0, y1):
    """Split row range [y0,y1) into (pa, pb, r0, r1): partitions [pa,pb)
    each contributing band-local rows [r0,r1)."""
    out = []
    ph = y0 // R_PER_PART
    pt = (y1 - 1) // R_PER_PART
    if ph == pt:
        out.append((ph, ph + 1, y0 % R_PER_PART, (y1 - 1) % R_PER_PART + 1))
        return out
    mid0 = ph
    if y0 % R_PER_PART:
        out.append((ph, ph + 1, y0 % R_PER_PART, R_PER_PART))
        mid0 = ph + 1
    mid1 = y1 // R_PER_PART
    if mid1 > mid0:
        out.append((mid0, mid1, 0, R_PER_PART))
    if y1 % R_PER_PART:
        out.append((pt, pt + 1, 0, y1 % R_PER_PART))
    return out


def build_matmul_plan():
    """Returns (specs_by_u, weight_mat).
    specs_by_u: {u: [(wvec_idx, r0, r1, x0, x1), ...]} in O12 order.
    weight_mat: np.float16 [128, n_w] per-partition weight columns."""
    terms = gen_terms()
    from collections import defaultdict
    specs_by_u = defaultdict(list)
    wkey_idx = {}
    wcols = []

    def wvec(pa, pb, val):
        key = (pa, pb, float(val))
        if key not in wkey_idx:
            col = np.zeros(128, np.float16)
            col[pa:pb] = np.float16(val)
            assert float(np.float16(val)) == float(val)
            wkey_idx[key] = len(wcols)
            wcols.append(col)
        return wkey_idx[key]

    for u, y0, y1, x0, x1, wgt in terms:
        for pa, pb, r0, r1 in _y_pieces(y0, y1):
            wi = wvec(pa, pb, wgt)
            nx = x1 - x0
            if (r1 - r0) * nx <= 512:
                specs_by_u[u].append((wi, r0, r1, x0, x1))
            else:
                for r in range(r0, r1):
                    # split x if needed (nx can be up to 512)
                    assert nx <= 512
                    specs_by_u[u].append((wi, r, r + 1, x0, x1))
    weight_mat = np.stack(wcols, axis=1).astype(np.float16)
    return dict(specs_by_u), weight_mat


_CACHE = {}


def _build_nc(n_w):
    import concourse.bacc as bacc
    import concourse.mybir as mybir
    import concourse.tile as tile

    dt = mybir.dt
    specs_by_u, _ = _CACHE["plan"]

    nc = bacc.Bacc("TRN2", target_bir_lowering=False, debug=False,
                   num_devices=NCORES)
    x_d = nc.dram_tensor("x", [128, XFREE], dt.float16, kind="ExternalInput")
    w_d = nc.dram_tensor("w", [128, n_w], dt.float16, kind="ExternalInput")
    out_d = nc.dram_tensor("out", [1, 512], dt.float32, kind="ExternalOutput")

    with tile.TileContext(nc) as tc:
        with (
            tc.tile_pool(name="xpool", bufs=1) as xpool,
            tc.tile_pool(name="cpool", bufs=2) as cpool,
            tc.tile_pool(name="xtpool", bufs=3) as xtpool,
            tc.tile_pool(name="misc", bufs=1) as misc,
            tc.tile_pool(name="psum", bufs=1, space="PSUM") as psum,
        ):
            X = xpool.tile([128, XFREE], dt.float16)
            Wt = misc.tile([128, n_w], dt.float16)
            acc = psum.tile([1, 512], dt.float32)
            res = misc.tile([1, 512], dt.float32)

            # band-granular loads (c0 bands first so compute starts early)
            Xv = X[:].rearrange("p (g ch f) -> p ch g f", g=2, ch=3)
            Sv = x_d[:].rearrange("p (g ch f) -> p ch g f", g=2, ch=3)
            for c in range(3):
                for g in range(2):
                    nc.sync.dma_start(Xv[:, c, g:g + 1], Sv[:, c, g:g + 1])
            nc.sync.dma_start(Wt[:], w_d[:])

            # X as [p][ch][g][band-row: 8][band-col: 516]
            X5 = X[:].rearrange("p (g ch row col) -> p ch g row col",
                                g=2, ch=3, row=8, col=BANDW)
            # X as [p][t: 6][band-row: 8][band-col: 516] (t = g*3 + ch)
            X4 = X[:].rearrange("p (t row col) -> p t row col",
                                t=6, row=8, col=BANDW)

            # heavy-correction offsets first; lightest last (tail exposure)
            offs = sorted(O12, key=lambda u: -len(specs_by_u[u]))

            # the very last (channel, offset) splits its xor + reductions
            # into rows 0-2 / row 3 so only the row-3 slice of the work is
            # exposed after the final isne
            u_last = offs[-1]
            specs_a, specs_b = [], []
            for wi, r0, r1, x0, x1 in specs_by_u[u_last]:
                if r1 <= 3:
                    specs_a.append((wi, r0, r1, x0, x1))
                elif r0 >= 3:
                    specs_b.append((wi, r0, r1, x0, x1))
                else:
                    specs_a.append((wi, r0, 3, x0, x1))
                    specs_b.append((wi, 3, r1, x0, x1))

            total = (sum(len(specs_by_u[u]) for u in O12) * 3
                     + len(specs_a) + len(specs_b) - len(specs_by_u[u_last]))
            state = {"nmm": 0, "cmp_insts": []}

            def emit_one(c, u, split_cmp=False):
                di, dj = u
                C = cpool.tile([128, 4096], dt.float16, tag="cmp")
                XT = xtpool.tile([128, 2048], dt.float16, tag="xt")
                if split_cmp:
                    # one compare per band so each op only waits on
                    # its own band's DMA (earlier start)
                    for g in range(2):
                        ctr = X5[:, c, g:g + 1, PAD:PAD + 4, PAD:PAD + W]
                        sft = X5[:, c, g:g + 1, PAD + di:PAD + 4 + di,
                                 PAD + dj:PAD + W + dj]
                        out = C[:, g * 2048:(g + 1) * 2048].rearrange(
                            "p (g r x) -> p g r x", g=1, r=4)
                        ins = nc.vector.tensor_tensor(
                            out, sft, ctr, mybir.AluOpType.is_lt)
                        state["cmp_insts"].append(ins.ins)
                else:
                    ctr = X5[:, c, :, PAD:PAD + 4, PAD:PAD + W]
                    sft = X5[:, c, :, PAD + di:PAD + 4 + di,
                             PAD + dj:PAD + W + dj]
                    Cv = C[:].rearrange("p (g r x) -> p g r x", g=2, r=4)
                    ins = nc.vector.tensor_tensor(
                        Cv, sft, ctr, mybir.AluOpType.is_lt)
                    state["cmp_insts"].append(ins.ins)
                nc.vector.tensor_tensor(
                    XT[:], C[:, 0:2048], C[:, 2048:4096],
                    mybir.AluOpType.not_equal)
                XT3 = XT[:].rearrange("p (r x) -> p r x", r=4)
                reduce_specs_1c(u, c, XT3)

            def emit_per_channel(u, split_first_cmp=False):
                for c in range(3):
                    emit_one(c, u, split_cmp=(split_first_cmp and c == 0))

            def reduce_specs_1c(u, c, XT3, specs=None):
                for wi, r0, r1, x0, x1 in (specs if specs is not None
                                           else specs_by_u[u]):
                    n = (r1 - r0) * (x1 - x0)
                    state["nmm"] += 1
                    nc.tensor.matmul(
                        acc[0:1, 0:n],
                        Wt[:, wi:wi + 1],
                        XT3[:, r0:r1, x0:x1],
                        start=(state["nmm"] == 1),
                        stop=(state["nmm"] == total),
                        skip_group_check=True,
                    )

            def emit_merged(u):
                # all channels in one compare (FD 12288) + one isne (6144)
                di, dj = u
                C = cpool.tile([128, 12288], dt.float16, tag="cmpm")
                XT = xtpool.tile([128, 6144], dt.float16, tag="xtm")
                ctr = X4[:, :, PAD:PAD + 4, PAD:PAD + W]
                sft = X4[:, :, PAD + di:PAD + 4 + di, PAD + dj:PAD + W + dj]
                Cv = C[:].rearrange("p (t r x) -> p t r x", t=6, r=4)
                nc.vector.tensor_tensor(Cv, sft, ctr, mybir.AluOpType.is_lt)
                nc.vector.tensor_tensor(XT[:], C[:, 0:6144], C[:, 6144:12288],
                                        mybir.AluOpType.not_equal)
                XT4 = XT[:].rearrange("p (c r x) -> p c r x", c=3, r=4)
                for c in range(3):
                    reduce_specs_1c(u, c, XT4[:, c])

            # phase A: first three offsets channel-major — all c0 work
            # first (only needs the first-loaded bands), then c1, c2;
            # c1/c2 DMAs complete under the c0 compute
            for c in range(3):
                for k, u in enumerate(offs[0:3]):
                    emit_one(c, u, split_cmp=(c == 0 and k == 0))
            # phase B: middle offsets channel-merged (fewer DVE ops)
            for u in offs[3:-1]:
                emit_merged(u)
            # phase C: lightest offset per-channel (small PE tail); the
            # final channel splits its xor by band-row so most reductions
            # overlap the last isne slice
            emit_one(0, u_last)
            emit_one(1, u_last)
            di, dj = u_last
            C = cpool.tile([128, 4096], dt.float16, tag="cmp")
            XT = xtpool.tile([128, 2048], dt.float16, tag="xt")
            ctr = X5[:, 2, :, PAD:PAD + 4, PAD:PAD + W]
            sft = X5[:, 2, :, PAD + di:PAD + 4 + di, PAD + dj:PAD + W + dj]
            Cv = C[:].rearrange("p (g r x) -> p g r x", g=2, r=4)
            nc.vector.tensor_tensor(Cv, sft, ctr, mybir.AluOpType.is_lt)
            XT3 = XT[:].rearrange("p (r x) -> p r x", r=4)
            nc.vector.tensor_tensor(XT[:, 0:1536], C[:, 0:1536],
                                    C[:, 2048:3584], mybir.AluOpType.not_equal)
            reduce_specs_1c(u_last, 2, XT3, specs=specs_a)
            nc.vector.tensor_tensor(XT[:, 1536:2048], C[:, 1536:2048],
                                    C[:, 3584:4096], mybir.AluOpType.not_equal)
            reduce_specs_1c(u_last, 2, XT3, specs=specs_b)

            assert state["nmm"] == total, (state["nmm"], total)
            nc.vector.tensor_copy(res[:], acc[:])
            nc.sync.dma_start(out_d[:], res[:])
    nc.compile()
    return nc


def _get_compiled():
    if "nc" not in _CACHE:
        _CACHE["plan"] = build_matmul_plan()
        # reorder baseline terms first per u: ensure first matmul is N=512
        specs_by_u, wmat = _CACHE["plan"]
        for u in O12:
            specs_by_u[u].sort(key=lambda s: -((s[2] - s[1]) * (s[4] - s[3])))
        _CACHE["plan"] = (specs_by_u, wmat)
        _CACHE["nc"] = _build_nc(wmat.shape[1])
    return _CACHE["nc"], _CACHE["plan"][1]


def host_prep(pred_i, gt_i):
    """pred_i, gt_i: f32 [3,512,512] -> banded fp16 [128, XFREE]:
    per band t (pred ch 0-2, gt ch 0-2), partition p holds padded rows
    4p..4p+8 (4 own rows + 2-row halos each side)."""
    p = np.pad(pred_i, ((0, 0), (PAD, PAD), (PAD, PAD)), mode="reflect")
    g = np.pad(gt_i, ((0, 0), (PAD, PAD), (PAD, PAD)), mode="reflect")
    both = np.concatenate([p, g], axis=0).astype(np.float16)  # [6,516,516]
    s = both.strides
    bands = np.lib.stride_tricks.as_strided(
        both, shape=(6, 128, BANDF), strides=(s[0], R_PER_PART * s[1], s[2]))
    return np.ascontiguousarray(
        bands.transpose(1, 0, 2).reshape(128, XFREE))


def kernel(pred, gt):
    from concourse.bass_utils import run_bass_kernel_spmd

    pred = np.asarray(pred, dtype=np.float32)
    gt = np.asarray(gt, dtype=np.float32)
    assert pred.shape == (NCORES, 3, H, W), pred.shape

    nc, wmat = _get_compiled()
    in_maps = [
        {"x": host_prep(pred[i], gt[i]), "w": wmat} for i in range(NCORES)
    ]
    r = run_bass_kernel_spmd(nc, in_maps, list(range(NCORES)))
    total = 0.0
    for i in range(NCORES):
        total += r.results[i]["out"].astype(np.float64).sum()
    mean = total / (NCORES * 3 * H * W)
    return np.float32(mean)


if __name__ == "__main__":
    # quick self-test vs numpy mirror of the reference
    rng = np.random.default_rng(1)
    pred = rng.random((8, 3, H, W), dtype=np.float32)
    gt = rng.random((8, 3, H, W), dtype=np.float32)

    def np_reference(pred, gt):
        pw = ((0, 0), (0, 0), (PAD, PAD), (PAD, PAD))
        pp = np.pad(pred, pw, mode="reflect")
        gp = np.pad(gt, pw, mode="reflect")
        ham = np.zeros_like(pred, dtype=np.float64)
        for i in range(5):
            for j in range(5):
                if i == PAD and j == PAD:
                    continue
                pc = (pp[:, :, i:i + H, j:j + W] < pred)
                gc = (gp[:, :, i:i + H, j:j + W] < gt)
                ham += (pc != gc)
        return ham.mean()

    expect = np_reference(pred, gt)
    got = kernel(pred, gt)
    rel = abs(float(got) - expect) / abs(expect)
    print(f"expect={expect:.9f} got={float(got):.9f} rel={rel:.3e}")
    print("OK" if rel < 2e-4 else "FAIL")
